# revision 32
# baseline (speedup 1.0000x reference)
"""Trainium2 Bass kernel for EnhancedGNN (3x GCNConv + mean-pool + FC), v6.

Self-contained: host-side sharding/layout prep + SPMD Bass/Tile program on 8
NeuronCores. See bottom for the `kernel(**inputs)` entry point.

Design (measured best on HW; ~3.3-3.7 ms vs 4.73 ms for the previous version):
  - Epilogues software-pipelined one group behind the accumulation chains, so
    each group's PE/ACT ping-pong (psum->sbuf copy, W GEMMs, relu) sits
    between the NEXT group's chunk matmuls in the in-order engine queues
    instead of stalling them (measured -0.7 ms). Two-group delay is worse.
  - The idx preload is split into 8 slice DMAs so early gathers start before
    the full 7.4 MB table lands.
  - Nodes degree-balanced across 8 cores; edges bucketed by (dst core, dst
    tile group, src block); scatter-add done as per-chunk one-hot matmuls
    accumulating in PSUM (one open accumulation group per 2KB bank -- a HW
    rule; each tile group's chain owns one bank, 6 chains pipelined).
  - gemm1 eliminated by linearity: layer 1 aggregates raw x (gathered from a
    host-prepared [x|x] bf16 tensor) and applies W1 *after* aggregation in
    the epilogue (A^T(xW1) == (A^T x)W1). No G1 materialization.
  - GRP=1 (128-wide one-hot S): wide-mode S builds at W=256 double the DVE
    cost (measured DVE-bound), and per-chunk tensor_scalar builds collapse
    the pipeline (12.4 ms) despite being faster in isolation.
  - Gather grain ~10 chunks (1280 descriptors, near the 1024-desc SWDGE ring)
    with 8 gather buffers in flight: measured 2x faster than 2-3 buffers and
    than >4K-descriptor gathers (ring-overflow stalls).
  - GCN norm folded into per-edge weights on host; biases fused into the
    epilogue activations; mean-pool via one-hot batch matmul + AllReduce.
"""

import os
import sys

import numpy as np

for _p in ("/opt/trn_rl_repo", "/root/.axon_site", "/root/.axon_site/_ro/pypackages"):
    if os.path.isdir(_p) and _p not in sys.path:
        sys.path.append(_p)

import ml_dtypes

BF16 = ml_dtypes.bfloat16
P = 128


def cdiv(a, b):
    return -(-a // b)


class Cfg:
    def __init__(self, n_nodes, n_edges, nc, tiles_pc, grp, nblk, n_graphs):
        self.N = n_nodes
        self.E = n_edges
        self.NC = nc
        self.T = tiles_pc
        self.GRP = grp
        self.NBLK = nblk
        self.G = n_graphs
        self.NPC = self.T * P
        self.TOTAL = self.NC * self.NPC
        self.BLK = self.TOTAL // self.NBLK
        assert self.T % self.GRP == 0
        assert self.TOTAL % self.NBLK == 0
        assert self.BLK <= 32768
        assert self.N % self.NC == 0
        assert self.N // self.NC <= self.NPC
        self.F = (64, 64, 128, 64)  # F0(in), F1, F2, F3
        # filled by host_prep:
        self.CHTS = None      # {(g,b,j): n_chunks}
        self.CBASE = None     # {(g,b,j): first chunk col}
        self.NCHUNK = None    # total chunks
        self.CHT_MAX = None
        self.CHTB_MAX = None  # max chunks in a merged (g,b) bucket
        self.HAS_B3 = False


FULL_CFG = dict(n_nodes=100000, n_edges=3200000, nc=8, tiles_pc=98,
                grp=int(os.environ.get("K_GRP", "1")),
                nblk=4, n_graphs=64)


# --------------------------------------------------------------------------
# Host-side prep: node assignment, edge bucketing, layout arrays.
# --------------------------------------------------------------------------

def host_prep(x, src, dst, edge_weight, batch, W1, b1, W2, b2, W3, b3, Wfc,
              bfc, cfg: Cfg):
    N, E, NC, T = cfg.N, cfg.E, cfg.NC, cfg.T
    NPC, TOTAL, NBLK, BLK, GRP = cfg.NPC, cfg.TOTAL, cfg.NBLK, cfg.BLK, cfg.GRP
    NGRP = T // GRP
    GR = GRP * P
    F0 = cfg.F[0]
    x = np.ascontiguousarray(np.asarray(x, np.float32))
    src = np.asarray(src).astype(np.int64)
    dst = np.asarray(dst).astype(np.int64)
    ew = np.asarray(edge_weight, np.float32)
    batch = np.asarray(batch).astype(np.int64)

    # ---- node -> (core, tile, p) assignment, degree balanced ----
    degc = np.bincount(dst, minlength=N)
    order = np.argsort(-degc, kind="stable")
    ranks = np.arange(N)
    core_of = np.empty(N, np.int64)
    rank_in_core = np.empty(N, np.int64)
    core_of[order] = ranks % NC
    rank_in_core[order] = ranks // NC
    row = rank_in_core // T
    col = rank_in_core % T
    tile = np.where(row % 2 == 0, col, T - 1 - col)
    p_in_tile = row
    assert p_in_tile.max() < P
    # G-row id: within (core, group): p * GRP + j so group writes are
    # per-partition contiguous
    g_i = tile // GRP
    j_i = tile % GRP
    grow = core_of * NPC + g_i * GR + p_in_tile * GRP + j_i

    # ---- self loops + GCN norm folded into edge weights (host) ----
    loop = np.arange(N, dtype=np.int64)
    src_f = np.concatenate([src, loop])
    dst_f = np.concatenate([dst, loop])
    ew_f = np.concatenate([ew, np.ones(N, np.float32)])
    deg = np.bincount(dst_f, weights=ew_f.astype(np.float64),
                      minlength=N).astype(np.float32)
    dinv = np.where(deg > 0, 1.0 / np.sqrt(deg), 0.0).astype(np.float32)
    norm = dinv[src_f] * ew_f * dinv[dst_f]

    # ---- edge bucketing by (dst core, group g, src block b) ----
    # dst position inside the S one-hot is group-relative (j*128 + p), so
    # tiles of a group share one bucket and one PSUM accumulation chain.
    assert GRP * P <= 512  # PSUM bank holds 512 fp32 columns
    e_core = core_of[dst_f]
    e_g = g_i[dst_f]
    e_pos = j_i[dst_f] * P + p_in_tile[dst_f]   # 0 .. GRP*128-1
    e_grow = grow[src_f]
    e_B = e_grow // BLK
    e_lidx = (e_grow % BLK).astype(np.int64)
    # block-major bucket order (g innermost): adjacent tiles' buckets for
    # the same source block are contiguous, so one dma_gather can span
    # several tiles' chunks (K_GM merge) without touching chain structure.
    key = (e_core * NBLK + e_B) * NGRP + e_g
    si = np.argsort(key, kind="stable")
    key_s = key[si]
    nbuck = NC * NGRP * NBLK
    bc = np.bincount(key_s, minlength=nbuck).reshape(NC, NBLK * NGRP)
    # per-core chunk counts must be IDENTICAL across cores for SPMD (one
    # program): use per-bucket max over cores.
    chts_flat = cdiv(bc, P).max(axis=0)  # [NBLK*NGRP]
    cbase_flat = np.zeros(chts_flat.size + 1, np.int64)
    np.cumsum(chts_flat, out=cbase_flat[1:])
    nchunk = int(cbase_flat[-1])
    cfg.NCHUNK = nchunk
    cfg.CHT_MAX = int(chts_flat.max())
    CHTS = {}
    CBASE = {}
    for b in range(NBLK):
        for g in range(NGRP):
            f = b * NGRP + g
            CHTS[(b, g)] = int(chts_flat[f])
            CBASE[(b, g)] = int(cbase_flat[f])
    cfg.CHTS = CHTS
    cfg.CBASE = CBASE
    cfg.CHTB_MAX = cfg.CHT_MAX

    # ---- slot assignment within buckets ----
    starts = np.zeros(nbuck + 1, np.int64)
    np.cumsum(bc.reshape(-1), out=starts[1:])
    slot = np.arange(E + N) - starts[key_s]
    core_b = key_s // (NBLK * NGRP)
    buck = key_s % (NBLK * NGRP)

    # idx / dstf / wf arrays, chunk-column layout
    idx_arr = np.zeros((NC, nchunk * P), np.int16)
    dstf = np.full((NC, P, nchunk), -1.0, np.float32)
    wff = np.zeros((NC, P, nchunk), np.float32)
    ccol = cbase_flat[buck] + slot // P     # global chunk column
    pp = slot % P
    idx_arr[core_b, ccol * P + pp] = e_lidx[si].astype(np.int16)
    dstf[core_b, pp, ccol] = e_pos[si].astype(np.float32)
    wff[core_b, pp, ccol] = norm[si]

    # 16-wrap the indices per chunk: slot e of chunk c -> [e%16, c*8 + e//16],
    # replicated x8 along partitions.
    idx16 = idx_arr.reshape(NC, nchunk, 8, 16).transpose(0, 3, 1, 2)
    idx16 = idx16.reshape(NC, 16, nchunk * 8)
    idx16 = np.ascontiguousarray(
        np.broadcast_to(idx16[:, None, :, :], (NC, 8, 16, nchunk * 8))
        .reshape(NC, P, nchunk * 8))

    # ---- batch one-hot source values (per tile col), pad -> -1 ----
    batchf = np.full((NC, P, T), -1.0, np.float32)
    batchf[core_of, p_in_tile, tile] = batch.astype(np.float32)

    # ---- features as [x|x] bf16 rows in grow order ----
    xdup = np.zeros((TOTAL, 2 * F0), np.float32)
    xdup[grow, :F0] = x
    xdup[grow, F0:] = x

    # ---- constants ----
    iota = np.tile(np.arange(GRP * P, dtype=np.float32)[None, :], (P, 1))
    ident64 = np.eye(64, dtype=np.float32)
    cnts = np.maximum(np.bincount(batch, minlength=cfg.G).astype(np.float32),
                      1.0)
    cinv = (1.0 / cnts).reshape(cfg.G, 1)

    W1 = np.asarray(W1, np.float32)
    W2 = np.asarray(W2, np.float32)
    W3 = np.asarray(W3, np.float32)
    b1 = np.asarray(b1, np.float32).reshape(-1)
    b2 = np.asarray(b2, np.float32).reshape(-1)
    b3 = np.asarray(b3, np.float32).reshape(-1)
    cfg.HAS_B3 = bool(np.any(b3 != 0.0))
    assert not cfg.HAS_B3, "nonzero b3 not supported in this kernel version"
    W3dup = np.concatenate([W3, W3], axis=1)          # [128, 128]

    xdup_bf = xdup.astype(BF16)
    per_core = []
    for c in range(NC):
        m = {
            "xdup": xdup_bf,
            "idx16": np.ascontiguousarray(idx16[c]),
            "dstf": np.ascontiguousarray(dstf[c]).astype(BF16),
            "wf": np.ascontiguousarray(wff[c]).astype(BF16),
            "dstf32": np.ascontiguousarray(dstf[c]),
            "wf32": np.ascontiguousarray(wff[c]),
            "batchf": np.ascontiguousarray(batchf[c]),
            "iota": iota.astype(BF16),
            "ident64": ident64,
            "cinv": cinv,
            "W1": W1.astype(BF16),
            "W2": W2.astype(BF16),
            "W3dup": W3dup.astype(BF16),
            "Wfc": np.asarray(Wfc, np.float32).reshape(cfg.F[3], 1),
            "b1c": b1.reshape(cfg.F[1], 1),
            "b2c": b2.reshape(cfg.F[2], 1),
            "bfcr": np.full((64, 1), np.float32(np.asarray(bfc).reshape(-1)[0])),
        }
        per_core.append(m)
    return per_core


# --------------------------------------------------------------------------
# Bass/Tile SPMD program
# --------------------------------------------------------------------------

def build_program(cfg: Cfg):
    import concourse.bacc as bacc
    import concourse.mybir as mybir
    import concourse.tile as tile

    dt = mybir.dt
    f32 = dt.float32
    bf16 = dt.bfloat16
    Alu = mybir.AluOpType
    Act = mybir.ActivationFunctionType

    NC, T, GRP, NBLK = cfg.NC, cfg.T, cfg.GRP, cfg.NBLK
    NPC, TOTAL, BLK = cfg.NPC, cfg.TOTAL, cfg.BLK
    G = cfg.G
    F0, F1, F2, F3 = cfg.F
    NGRP = T // GRP
    GR = GRP * P
    NCHUNK = cfg.NCHUNK
    CHTS, CBASE = cfg.CHTS, cfg.CBASE
    CHTB_MAX = cfg.CHTB_MAX
    GM = int(os.environ.get("K_GM", "2"))   # tiles merged per dma_gather
    SEGM = max(
        sum(CHTS[(b, g)] for g in range(g0, min(g0 + GM, NGRP)))
        for g0 in range(0, NGRP, GM) for b in range(NBLK))
    GS_BUFS = int(os.environ.get("K_GBUFS", "8"))
    S_BUFS = int(os.environ.get("K_SBUFS", "8"))
    sbuild = os.environ.get("K_SBUILD", "wide")
    skips = set(filter(None, os.environ.get("K_SKIP", "").split(",")))

    nq = int(os.environ.get("K_QUEUES", "4"))
    scratch = int(os.environ.get("K_SCRATCH", "16384"))
    nc = bacc.Bacc("TRN2", target_bir_lowering=False, debug=False,
                   enable_asserts=False, num_devices=NC,
                   num_swdge_queues=nq, dynamic_dma_scratch_size=scratch)
    _qctr = [0]

    def next_q():
        q = _qctr[0] % nq
        _qctr[0] += 1
        return q

    def inp(name, shape, dtype=f32):
        return nc.dram_tensor(name, list(shape), dtype, kind="ExternalInput")

    xdup = inp("xdup", (TOTAL, 128), bf16)
    idx16 = inp("idx16", (P, NCHUNK * 8), dt.int16)
    dstf = inp("dstf", (P, NCHUNK), bf16)
    wf = inp("wf", (P, NCHUNK), bf16)
    dstf32 = inp("dstf32", (P, NCHUNK))
    wf32 = inp("wf32", (P, NCHUNK))
    batchf = inp("batchf", (P, T))
    iota_in = inp("iota", (P, GRP * P), bf16)
    ident64_in = inp("ident64", (64, 64))
    cinv_in = inp("cinv", (G, 1))
    W1_in = inp("W1", (F0, F1), bf16)
    W2_in = inp("W2", (F1, F2), bf16)
    W3_in = inp("W3dup", (F2, 128), bf16)
    Wfc_in = inp("Wfc", (F3, 1))
    b1_in = inp("b1c", (F1, 1))
    b2_in = inp("b2c", (F2, 1))
    bfc_in = inp("bfcr", (64, 1))
    out_t = nc.dram_tensor("out", [64, 1], f32, kind="ExternalOutput")

    rg = [list(range(NC))]

    with tile.TileContext(nc) as tc:
        import contextlib
        ctx = contextlib.ExitStack()
        with ctx:
            dram = ctx.enter_context(tc.tile_pool(name="dram", bufs=1, space="DRAM"))
            pers = ctx.enter_context(tc.tile_pool(name="pers", bufs=1))
            sb2 = ctx.enter_context(tc.tile_pool(name="sb2", bufs=2))
            sb3 = ctx.enter_context(tc.tile_pool(name="sb3", bufs=3))
            spool = ctx.enter_context(tc.tile_pool(name="spool", bufs=4))
            gpool = ctx.enter_context(tc.tile_pool(name="gpool", bufs=GS_BUFS))
            # GRP concurrent per-tile accumulation chains (1 PSUM bank each);
            # extra slots pipeline the next group's chains. Remaining banks
            # host the transient psums (epilogue GEMMs, pooling, FC).
            n_gemm = 1 if GRP > 5 else 2
            abufs = int(os.environ.get("K_ABUFS", "0")) or (8 - n_gemm)
            agg_ps = ctx.enter_context(tc.tile_pool(name="agg_ps", bufs=abufs,
                                                    space="PSUM"))
            gemm_ps = ctx.enter_context(tc.tile_pool(name="gemm_ps",
                                                     bufs=n_gemm,
                                                     space="PSUM"))

            # ---------- DRAM intermediates ----------
            G2_shard = dram.tile([NPC, 128], bf16, name="G2_shard")
            G3_shard = dram.tile([NPC, 128], bf16, name="G3_shard")
            pool_in = dram.tile([64, F3], f32, name="pool_in")

            # ---------- persistent SBUF ----------
            # split the big idx preload so early gathers can start before
            # the whole table has landed (subtile deps gate per-slice)
            idx_sb = pers.tile([P, NCHUNK * 8], dt.int16, name="idx_sb")
            qc = cdiv(NCHUNK, 8)
            for i in range(8):
                lo, hi = i * qc * 8, min(NCHUNK, (i + 1) * qc) * 8
                if lo < hi:
                    nc.sync.dma_start(idx_sb[:, lo:hi], idx16[:, lo:hi])
            if sbuild == "ts":
                dstf_sb = pers.tile([P, NCHUNK], f32, name="dstf_sb")
                wf_sb = pers.tile([P, NCHUNK], f32, name="wf_sb")
                nc.sync.dma_start(dstf_sb[:], dstf32[:])
                nc.sync.dma_start(wf_sb[:], wf32[:])
            else:
                dstf_sb = pers.tile([P, NCHUNK], bf16, name="dstf_sb")
                wf_sb = pers.tile([P, NCHUNK], bf16, name="wf_sb")
                nc.sync.dma_start(dstf_sb[:], dstf[:])
                nc.sync.dma_start(wf_sb[:], wf[:])
            iota_sb = pers.tile([P, GRP * P], bf16, name="iota_sb")
            ident64_sb = pers.tile([64, 64], f32, name="ident64_sb")
            cinv_sb = pers.tile([G, 1], f32, name="cinv_sb")
            batchf_sb = pers.tile([P, T], f32, name="batchf_sb")
            nc.sync.dma_start(iota_sb[:], iota_in[:])
            nc.sync.dma_start(ident64_sb[:], ident64_in[:])
            nc.sync.dma_start(cinv_sb[:], cinv_in[:])
            nc.sync.dma_start(batchf_sb[:], batchf[:])
            W1_sb = pers.tile([F0, F1], bf16, name="W1_sb")
            W2_sb = pers.tile([F1, F2], bf16, name="W2_sb")
            W3_sb = pers.tile([F2, 128], bf16, name="W3_sb")
            Wfc_sb = pers.tile([F3, 1], f32, name="Wfc_sb")
            nc.sync.dma_start(W1_sb[:], W1_in[:])
            nc.sync.dma_start(W2_sb[:], W2_in[:])
            nc.sync.dma_start(W3_sb[:], W3_in[:])
            nc.sync.dma_start(Wfc_sb[:], Wfc_in[:])
            b1_sb = pers.tile([F1, 1], f32, name="b1_sb")
            b2_sb = pers.tile([F2, 1], f32, name="b2_sb")
            bfc_sb = pers.tile([64, 1], f32, name="bfc_sb")
            nc.sync.dma_start(b1_sb[:], b1_in[:])
            nc.sync.dma_start(b2_sb[:], b2_in[:])
            nc.sync.dma_start(bfc_sb[:], bfc_in[:])
            pool_sb = pers.tile([64, F3], f32, name="pool_sb")
            nc.vector.memset(pool_sb[:], 0.0)

            # ================= aggregation layer ==========================
            def agg_layer(li):
                """li=0: consume xdup, produce X2T->G2_shard (agg then W1,W2)
                   li=1: consume G2_full, produce X3T->G3_shard (transposed)
                   li=2: consume G3_full, produce pooled partials (normal)."""
                g_src = (xdup, G2_full, G3_full)[li]
                FI = (64, 128, 64)[li]       # real feature width of G rows
                transposed = li != 2
                W = GRP * P                  # group-relative position width

                def epilogue(g, apsg, aps):
                    if li == 0:
                        # xa = (A^T x)^T; h1 = relu(W1^T xa + b1);
                        # G2 = h1^T @ W2  (group-wide until the W2 GEMM)
                        stage = sb3.tile([P, GRP * F2], bf16, name="g2st",
                                         tag="g2st")
                        xa = spool.tile([F0, W], bf16, name="xa", tag="xa")
                        nc.scalar.copy(xa[:], apsg[:])
                        ps1 = gemm_ps.tile([F1, W], f32, name="ps1",
                                           tag="gps")
                        nc.tensor.matmul(ps1[:], lhsT=W1_sb[:], rhs=xa[:],
                                         start=True, stop=True)
                        x2t = spool.tile([F1, W], bf16, name="x2t",
                                         tag="x2t")
                        nc.scalar.activation(x2t[:], ps1[:], Act.Relu,
                                             bias=b1_sb[:, 0:1])
                        for j in range(GRP):
                            ps2 = gemm_ps.tile([P, F2], f32, name="ps2",
                                               tag="gps")
                            nc.tensor.matmul(ps2[:],
                                             lhsT=x2t[:, j * P:(j + 1) * P],
                                             rhs=W2_sb[:],
                                             start=True, stop=True)
                            nc.scalar.copy(stage[:, j * F2:(j + 1) * F2],
                                           ps2[:])
                        rows = G2_shard[g * GR:(g + 1) * GR, :]
                        nc.sync.dma_start(
                            rows.rearrange("(p j) f -> p j f", j=GRP),
                            stage[:].rearrange("p (j f) -> p j f", j=GRP))
                    elif li == 1:
                        stage = sb3.tile([P, GRP * 128], bf16, name="g3st",
                                         tag="g3st")
                        x3t = spool.tile([F2, W], bf16, name="x3t",
                                         tag="x3t")
                        nc.scalar.activation(x3t[:], apsg[:], Act.Relu,
                                             bias=b2_sb[:, 0:1])
                        for j in range(GRP):
                            ps3 = gemm_ps.tile([P, 128], f32, name="ps3",
                                               tag="gps")
                            nc.tensor.matmul(ps3[:],
                                             lhsT=x3t[:, j * P:(j + 1) * P],
                                             rhs=W3_sb[:],
                                             start=True, stop=True)
                            nc.scalar.copy(stage[:, j * 128:(j + 1) * 128],
                                           ps3[:])
                        rows = G3_shard[g * GR:(g + 1) * GR, :]
                        nc.sync.dma_start(
                            rows.rearrange("(p j) f -> p j f", j=GRP),
                            stage[:].rearrange("p (j f) -> p j f", j=GRP))
                    else:
                        pp = gemm_ps.tile([64, F3], f32, name="pp", tag="gps")
                        for j in range(GRP):
                            x4 = spool.tile([P, F3], bf16, name="x4", tag="x4")
                            nc.scalar.activation(x4[:], aps[j][:], Act.Relu)
                            t = g * GRP + j
                            Bt = spool.tile([P, 64], bf16, name="Bt", tag="Bt")
                            nc.vector.tensor_scalar(
                                Bt[:], iota_sb[:, :64],
                                batchf_sb[:, t:t + 1], None, Alu.is_equal)
                            nc.tensor.matmul(pp[:], lhsT=Bt[:], rhs=x4[:],
                                             start=(j == 0),
                                             stop=(j == GRP - 1))
                        nc.vector.tensor_tensor(out=pool_sb[:],
                                                in0=pool_sb[:], in1=pp[:],
                                                op=Alu.add)

                pend = []
                edelay = int(os.environ.get("K_EDELAY", "1"))
                for g0 in range(0, NGRP, GM):
                    gs = list(range(g0, min(g0 + GM, NGRP)))
                    # accumulation chains: one W-wide chain per group for
                    # transposed layers; GRP narrow chains for the last layer.
                    chains = {}
                    fcol = {}
                    lcol = {}
                    for g in gs:
                        if transposed:
                            chains[g] = (agg_ps.tile([FI, W], f32,
                                                     name="apsT", tag="aps"),
                                         None)
                        else:
                            chains[g] = (None,
                                         [agg_ps.tile([P, F3], f32,
                                                      name="aps", tag="aps")
                                          for _ in range(GRP)])
                        bs = [b for b in range(NBLK) if CHTS[(b, g)] > 0]
                        assert bs, g
                        fcol[g] = CBASE[(bs[0], g)]
                        lcol[g] = CBASE[(bs[-1], g)] + CHTS[(bs[-1], g)] - 1
                    for b in range(NBLK):
                        mcht = sum(CHTS[(b, g)] for g in gs)
                        if mcht == 0:
                            continue
                        cb0 = CBASE[(b, gs[0])]   # gs contiguous in chunk cols
                        gb = gpool.tile([P, SEGM, 128], bf16,
                                        name="gb", tag="gb")
                        if "gather" not in skips:
                            nc.gpsimd.dma_gather(
                                gb[:, :mcht, :],
                                g_src[b * BLK:(b + 1) * BLK, :],
                                idx_sb[:, cb0 * 8:(cb0 + mcht) * 8],
                                mcht * P, mcht * P, 128, single_packet=False,
                                queue_num=next_q())
                        S = spool.tile([P, SEGM, W], bf16,
                                       name="S", tag="S", bufs=S_BUFS)
                        if "svec" not in skips:
                            wcol = wf_sb[:, cb0:cb0 + mcht]
                            gview = gb[:, :mcht, :FI]
                            nc.vector.tensor_tensor(
                                out=gview, in0=gview,
                                in1=wcol[:, :, None].broadcast_to(
                                    [P, mcht, FI]),
                                op=Alu.mult)
                            nc.vector.tensor_tensor(
                                out=S[:, :mcht, :],
                                in0=iota_sb[:, None, :].broadcast_to(
                                    [P, mcht, W]),
                                in1=dstf_sb[:, cb0:cb0 + mcht]
                                [:, :, None].broadcast_to([P, mcht, W]),
                                op=Alu.is_equal)
                        if "mm" in skips:
                            continue
                        for g in gs:
                            apsg, aps = chains[g]
                            cg0 = CBASE[(b, g)]
                            for c in range(cg0, cg0 + CHTS[(b, g)]):
                                k = c - cb0
                                first = c == fcol[g]
                                last = c == lcol[g]
                                if transposed:
                                    nc.tensor.matmul(
                                        apsg[:], lhsT=gb[:, k, :FI],
                                        rhs=S[:, k, :],
                                        start=first, stop=last)
                                else:
                                    for j in range(GRP):
                                        nc.tensor.matmul(
                                            aps[j][:],
                                            lhsT=S[:, k, j * P:(j + 1) * P],
                                            rhs=gb[:, k, :FI],
                                            start=first, stop=last)
                    # ---- epilogue, delayed so the next groups' chunk
                    # matmuls sit between dependent epilogue steps in the
                    # in-order engine queues (no PE/ACT ping-pong stalls).
                    if "mm" in skips:
                        continue
                    for g in gs:
                        pend.append((g,) + chains[g])
                        if len(pend) > edelay:
                            epilogue(*pend.pop(0))
                while pend:
                    epilogue(*pend.pop(0))

            # ================= execution =================
            STAGES = ["prep", "agg1pre", "agg1", "agg2pre", "agg2",
                      "agg3", "full"]
            cut = os.environ.get("K_CUT", "full") or "full"
            cut_idx = STAGES.index(cut)

            reps = int(os.environ.get("K_REPS", "1"))
            for _rep in range(reps):
                G2_full = dram.tile([TOTAL, 128], bf16,
                                    name=f"G2_full_{_rep}",
                                    addr_space="Shared")
                G3_full = dram.tile([TOTAL, 128], bf16,
                                    name=f"G3_full_{_rep}",
                                    addr_space="Shared")
                pool_out = dram.tile([64, F3], f32, name=f"pool_out_{_rep}",
                                     addr_space="Shared")
                if _rep > 0:
                    nc.vector.memset(pool_sb[:], 0.0)
                if cut_idx >= 1:
                    agg_layer(0)
                if cut_idx >= 2:
                    nc.gpsimd.collective_compute(
                        "AllGather", Alu.bypass, ins=[G2_shard.opt()],
                        outs=[G2_full.opt()], replica_groups=rg)
                if cut_idx >= 3:
                    agg_layer(1)
                if cut_idx >= 4:
                    nc.gpsimd.collective_compute(
                        "AllGather", Alu.bypass, ins=[G3_shard.opt()],
                        outs=[G3_full.opt()], replica_groups=rg)
                if cut_idx >= 5:
                    agg_layer(2)
                if cut_idx < 6:
                    continue

                # ---- pool + FC ----
                nc.sync.dma_start(pool_in[:], pool_sb[:])
                nc.gpsimd.collective_compute(
                    "AllReduce", Alu.add, ins=[pool_in.opt()],
                    outs=[pool_out.opt()], replica_groups=rg)
                pr = sb2.tile([64, F3], f32, name="pr", tag="pr")
                nc.sync.dma_start(pr[:], pool_out[:])
                tsp = gemm_ps.tile([F3, 64], f32, name="tsp", tag="gps")
                nc.tensor.transpose(tsp[:], pr[:], ident64_sb[:])
                sT = sb2.tile([F3, 64], f32, name="sT", tag="sT")
                nc.vector.tensor_copy(sT[:], tsp[:])
                fps = gemm_ps.tile([64, 1], f32, name="fps", tag="gps")
                nc.tensor.matmul(fps[:], lhsT=sT[:], rhs=Wfc_sb[:], start=True,
                                 stop=True)
                res = sb2.tile([64, 1], f32, name="res", tag="res")
                nc.vector.tensor_scalar(res[:], fps[:], cinv_sb[:], bfc_sb[:],
                                        Alu.mult, op1=Alu.add)
                nc.sync.dma_start(out_t[:], res[:])

    nc.compile()
    return nc


# --------------------------------------------------------------------------
# Entry point
# --------------------------------------------------------------------------

_PROGRAM_CACHE = {}


def kernel(x, src, dst, edge_weight, batch, W1, b1, W2, b2, W3, b3, Wfc, bfc):
    from concourse.bass_utils import run_bass_kernel_spmd

    cfg = Cfg(**FULL_CFG)
    per_core = host_prep(x, src, dst, edge_weight, batch, W1, b1, W2, b2, W3,
                         b3, Wfc, bfc, cfg)
    key = (tuple(sorted(cfg.CHTS.items())), cfg.HAS_B3)
    if key not in _PROGRAM_CACHE:
        _PROGRAM_CACHE[key] = build_program(cfg)
    nc = _PROGRAM_CACHE[key]
    res = run_bass_kernel_spmd(nc, per_core, list(range(cfg.NC)))
    out = np.asarray(res.results[0]["out"], np.float32).reshape(cfg.G, 1)
    return out


# revision 33
# speedup vs baseline: 1.0155x; 1.0155x over previous
"""Trainium2 Bass kernel for EnhancedGNN (3x GCNConv + mean-pool + FC), v6.

Self-contained: host-side sharding/layout prep + SPMD Bass/Tile program on 8
NeuronCores. See bottom for the `kernel(**inputs)` entry point.

Design (measured best on HW; ~3.3-3.7 ms vs 4.73 ms for the previous version):
  - Epilogues software-pipelined one group behind the accumulation chains, so
    each group's PE/ACT ping-pong (psum->sbuf copy, W GEMMs, relu) sits
    between the NEXT group's chunk matmuls in the in-order engine queues
    instead of stalling them (measured -0.7 ms). Two-group delay is worse.
  - The idx preload is split into 8 slice DMAs so early gathers start before
    the full 7.4 MB table lands.
  - Nodes degree-balanced across 8 cores; edges bucketed by (dst core, dst
    tile group, src block); scatter-add done as per-chunk one-hot matmuls
    accumulating in PSUM (one open accumulation group per 2KB bank -- a HW
    rule; each tile group's chain owns one bank, 6 chains pipelined).
  - gemm1 eliminated by linearity: layer 1 aggregates raw x (gathered from a
    host-prepared [x|x] bf16 tensor) and applies W1 *after* aggregation in
    the epilogue (A^T(xW1) == (A^T x)W1). No G1 materialization.
  - GRP=1 (128-wide one-hot S): wide-mode S builds at W=256 double the DVE
    cost (measured DVE-bound), and per-chunk tensor_scalar builds collapse
    the pipeline (12.4 ms) despite being faster in isolation.
  - Gather grain ~10 chunks (1280 descriptors, near the 1024-desc SWDGE ring)
    with 8 gather buffers in flight: measured 2x faster than 2-3 buffers and
    than >4K-descriptor gathers (ring-overflow stalls).
  - GCN norm folded into per-edge weights on host; biases fused into the
    epilogue activations; mean-pool via one-hot batch matmul + AllReduce.
"""

import os
import sys

import numpy as np

for _p in ("/opt/trn_rl_repo", "/root/.axon_site", "/root/.axon_site/_ro/pypackages"):
    if os.path.isdir(_p) and _p not in sys.path:
        sys.path.append(_p)

import ml_dtypes

BF16 = ml_dtypes.bfloat16
P = 128


def cdiv(a, b):
    return -(-a // b)


class Cfg:
    def __init__(self, n_nodes, n_edges, nc, tiles_pc, grp, nblk, n_graphs):
        self.N = n_nodes
        self.E = n_edges
        self.NC = nc
        self.T = tiles_pc
        self.GRP = grp
        self.NBLK = nblk
        self.G = n_graphs
        self.NPC = self.T * P
        self.TOTAL = self.NC * self.NPC
        self.BLK = self.TOTAL // self.NBLK
        assert self.T % self.GRP == 0
        assert self.TOTAL % self.NBLK == 0
        assert self.BLK <= 32768
        assert self.N % self.NC == 0
        assert self.N // self.NC <= self.NPC
        self.F = (64, 64, 128, 64)  # F0(in), F1, F2, F3
        # filled by host_prep:
        self.CHTS = None      # {(g,b,j): n_chunks}
        self.CBASE = None     # {(g,b,j): first chunk col}
        self.NCHUNK = None    # total chunks
        self.CHT_MAX = None
        self.CHTB_MAX = None  # max chunks in a merged (g,b) bucket
        self.HAS_B3 = False


FULL_CFG = dict(n_nodes=100000, n_edges=3200000, nc=8, tiles_pc=98,
                grp=int(os.environ.get("K_GRP", "1")),
                nblk=4, n_graphs=64)


# --------------------------------------------------------------------------
# Host-side prep: node assignment, edge bucketing, layout arrays.
# --------------------------------------------------------------------------

def host_prep(x, src, dst, edge_weight, batch, W1, b1, W2, b2, W3, b3, Wfc,
              bfc, cfg: Cfg):
    N, E, NC, T = cfg.N, cfg.E, cfg.NC, cfg.T
    NPC, TOTAL, NBLK, BLK, GRP = cfg.NPC, cfg.TOTAL, cfg.NBLK, cfg.BLK, cfg.GRP
    NGRP = T // GRP
    GR = GRP * P
    F0 = cfg.F[0]
    x = np.ascontiguousarray(np.asarray(x, np.float32))
    src = np.asarray(src).astype(np.int64)
    dst = np.asarray(dst).astype(np.int64)
    ew = np.asarray(edge_weight, np.float32)
    batch = np.asarray(batch).astype(np.int64)

    # ---- node -> (core, tile, p) assignment, degree balanced ----
    degc = np.bincount(dst, minlength=N)
    order = np.argsort(-degc, kind="stable")
    ranks = np.arange(N)
    core_of = np.empty(N, np.int64)
    rank_in_core = np.empty(N, np.int64)
    core_of[order] = ranks % NC
    rank_in_core[order] = ranks // NC
    row = rank_in_core // T
    col = rank_in_core % T
    tile = np.where(row % 2 == 0, col, T - 1 - col)
    p_in_tile = row
    assert p_in_tile.max() < P
    # G-row id: within (core, group): p * GRP + j so group writes are
    # per-partition contiguous
    g_i = tile // GRP
    j_i = tile % GRP
    grow = core_of * NPC + g_i * GR + p_in_tile * GRP + j_i

    # ---- self loops + GCN norm folded into edge weights (host) ----
    loop = np.arange(N, dtype=np.int64)
    src_f = np.concatenate([src, loop])
    dst_f = np.concatenate([dst, loop])
    ew_f = np.concatenate([ew, np.ones(N, np.float32)])
    deg = np.bincount(dst_f, weights=ew_f.astype(np.float64),
                      minlength=N).astype(np.float32)
    dinv = np.where(deg > 0, 1.0 / np.sqrt(deg), 0.0).astype(np.float32)
    norm = dinv[src_f] * ew_f * dinv[dst_f]

    # ---- edge bucketing by (dst core, group g, src block b) ----
    # dst position inside the S one-hot is group-relative (j*128 + p), so
    # tiles of a group share one bucket and one PSUM accumulation chain.
    assert GRP * P <= 512  # PSUM bank holds 512 fp32 columns
    e_core = core_of[dst_f]
    e_g = g_i[dst_f]
    e_pos = j_i[dst_f] * P + p_in_tile[dst_f]   # 0 .. GRP*128-1
    e_grow = grow[src_f]
    e_B = e_grow // BLK
    e_lidx = (e_grow % BLK).astype(np.int64)
    # block-major bucket order (g innermost): adjacent tiles' buckets for
    # the same source block are contiguous, so one dma_gather can span
    # several tiles' chunks (K_GM merge) without touching chain structure.
    key = (e_core * NBLK + e_B) * NGRP + e_g
    si = np.argsort(key, kind="stable")
    key_s = key[si]
    nbuck = NC * NGRP * NBLK
    bc = np.bincount(key_s, minlength=nbuck).reshape(NC, NBLK * NGRP)
    # per-core chunk counts must be IDENTICAL across cores for SPMD (one
    # program): use per-bucket max over cores.
    chts_flat = cdiv(bc, P).max(axis=0)  # [NBLK*NGRP]
    cbase_flat = np.zeros(chts_flat.size + 1, np.int64)
    np.cumsum(chts_flat, out=cbase_flat[1:])
    nchunk = int(cbase_flat[-1])
    cfg.NCHUNK = nchunk
    cfg.CHT_MAX = int(chts_flat.max())
    CHTS = {}
    CBASE = {}
    for b in range(NBLK):
        for g in range(NGRP):
            f = b * NGRP + g
            CHTS[(b, g)] = int(chts_flat[f])
            CBASE[(b, g)] = int(cbase_flat[f])
    cfg.CHTS = CHTS
    cfg.CBASE = CBASE
    cfg.CHTB_MAX = cfg.CHT_MAX

    # ---- slot assignment within buckets ----
    starts = np.zeros(nbuck + 1, np.int64)
    np.cumsum(bc.reshape(-1), out=starts[1:])
    slot = np.arange(E + N) - starts[key_s]
    core_b = key_s // (NBLK * NGRP)
    buck = key_s % (NBLK * NGRP)

    # idx / dstf / wf arrays, chunk-column layout
    idx_arr = np.zeros((NC, nchunk * P), np.int16)
    dstf = np.full((NC, P, nchunk), -1.0, np.float32)
    wff = np.zeros((NC, P, nchunk), np.float32)
    ccol = cbase_flat[buck] + slot // P     # global chunk column
    pp = slot % P
    idx_arr[core_b, ccol * P + pp] = e_lidx[si].astype(np.int16)
    dstf[core_b, pp, ccol] = e_pos[si].astype(np.float32)
    wff[core_b, pp, ccol] = norm[si]

    # 16-wrap the indices per chunk: slot e of chunk c -> [e%16, c*8 + e//16],
    # replicated x8 along partitions.
    idx16 = idx_arr.reshape(NC, nchunk, 8, 16).transpose(0, 3, 1, 2)
    idx16 = idx16.reshape(NC, 16, nchunk * 8)
    idx16 = np.ascontiguousarray(
        np.broadcast_to(idx16[:, None, :, :], (NC, 8, 16, nchunk * 8))
        .reshape(NC, P, nchunk * 8))

    # ---- batch one-hot source values (per tile col), pad -> -1 ----
    batchf = np.full((NC, P, T), -1.0, np.float32)
    batchf[core_of, p_in_tile, tile] = batch.astype(np.float32)

    # ---- features as [x|x] bf16 rows in grow order ----
    xdup = np.zeros((TOTAL, 2 * F0), np.float32)
    xdup[grow, :F0] = x
    xdup[grow, F0:] = x

    # ---- constants ----
    iota = np.tile(np.arange(GRP * P, dtype=np.float32)[None, :], (P, 1))
    ident64 = np.eye(64, dtype=np.float32)
    cnts = np.maximum(np.bincount(batch, minlength=cfg.G).astype(np.float32),
                      1.0)
    cinv = (1.0 / cnts).reshape(cfg.G, 1)

    W1 = np.asarray(W1, np.float32)
    W2 = np.asarray(W2, np.float32)
    W3 = np.asarray(W3, np.float32)
    b1 = np.asarray(b1, np.float32).reshape(-1)
    b2 = np.asarray(b2, np.float32).reshape(-1)
    b3 = np.asarray(b3, np.float32).reshape(-1)
    cfg.HAS_B3 = bool(np.any(b3 != 0.0))
    assert not cfg.HAS_B3, "nonzero b3 not supported in this kernel version"
    W3dup = np.concatenate([W3, W3], axis=1)          # [128, 128]

    xdup_bf = xdup.astype(BF16)
    per_core = []
    for c in range(NC):
        m = {
            "xdup": xdup_bf,
            "idx16": np.ascontiguousarray(idx16[c]),
            "dstf": np.ascontiguousarray(dstf[c]).astype(BF16),
            "wf": np.ascontiguousarray(wff[c]).astype(BF16),
            "dstf32": np.ascontiguousarray(dstf[c]),
            "wf32": np.ascontiguousarray(wff[c]),
            "batchf": np.ascontiguousarray(batchf[c]),
            "iota": iota.astype(BF16),
            "ident64": ident64,
            "cinv": cinv,
            "W1": W1.astype(BF16),
            "W2": W2.astype(BF16),
            "W3dup": W3dup.astype(BF16),
            "Wfc": np.asarray(Wfc, np.float32).reshape(cfg.F[3], 1),
            "b1c": b1.reshape(cfg.F[1], 1),
            "b2c": b2.reshape(cfg.F[2], 1),
            "bfcr": np.full((64, 1), np.float32(np.asarray(bfc).reshape(-1)[0])),
        }
        per_core.append(m)
    return per_core


# --------------------------------------------------------------------------
# Bass/Tile SPMD program
# --------------------------------------------------------------------------

def build_program(cfg: Cfg):
    import concourse.bacc as bacc
    import concourse.mybir as mybir
    import concourse.tile as tile

    dt = mybir.dt
    f32 = dt.float32
    bf16 = dt.bfloat16
    Alu = mybir.AluOpType
    Act = mybir.ActivationFunctionType

    NC, T, GRP, NBLK = cfg.NC, cfg.T, cfg.GRP, cfg.NBLK
    NPC, TOTAL, BLK = cfg.NPC, cfg.TOTAL, cfg.BLK
    G = cfg.G
    F0, F1, F2, F3 = cfg.F
    NGRP = T // GRP
    GR = GRP * P
    NCHUNK = cfg.NCHUNK
    CHTS, CBASE = cfg.CHTS, cfg.CBASE
    CHTB_MAX = cfg.CHTB_MAX
    GM = int(os.environ.get("K_GM", "1"))   # tiles merged per dma_gather
    SEGM = max(
        sum(CHTS[(b, g)] for g in range(g0, min(g0 + GM, NGRP)))
        for g0 in range(0, NGRP, GM) for b in range(NBLK))
    GS_BUFS = int(os.environ.get("K_GBUFS", "8"))
    S_BUFS = int(os.environ.get("K_SBUFS", "8"))
    sbuild = os.environ.get("K_SBUILD", "wide")
    skips = set(filter(None, os.environ.get("K_SKIP", "").split(",")))

    nq = int(os.environ.get("K_QUEUES", "4"))
    scratch = int(os.environ.get("K_SCRATCH", "16384"))
    nc = bacc.Bacc("TRN2", target_bir_lowering=False, debug=False,
                   enable_asserts=False, num_devices=NC,
                   num_swdge_queues=nq, dynamic_dma_scratch_size=scratch)
    _qctr = [0]

    def next_q():
        q = _qctr[0] % nq
        _qctr[0] += 1
        return q

    def inp(name, shape, dtype=f32):
        return nc.dram_tensor(name, list(shape), dtype, kind="ExternalInput")

    xdup = inp("xdup", (TOTAL, 128), bf16)
    idx16 = inp("idx16", (P, NCHUNK * 8), dt.int16)
    dstf = inp("dstf", (P, NCHUNK), bf16)
    wf = inp("wf", (P, NCHUNK), bf16)
    dstf32 = inp("dstf32", (P, NCHUNK))
    wf32 = inp("wf32", (P, NCHUNK))
    batchf = inp("batchf", (P, T))
    iota_in = inp("iota", (P, GRP * P), bf16)
    ident64_in = inp("ident64", (64, 64))
    cinv_in = inp("cinv", (G, 1))
    W1_in = inp("W1", (F0, F1), bf16)
    W2_in = inp("W2", (F1, F2), bf16)
    W3_in = inp("W3dup", (F2, 128), bf16)
    Wfc_in = inp("Wfc", (F3, 1))
    b1_in = inp("b1c", (F1, 1))
    b2_in = inp("b2c", (F2, 1))
    bfc_in = inp("bfcr", (64, 1))
    out_t = nc.dram_tensor("out", [64, 1], f32, kind="ExternalOutput")

    rg = [list(range(NC))]

    with tile.TileContext(nc) as tc:
        import contextlib
        ctx = contextlib.ExitStack()
        with ctx:
            dram = ctx.enter_context(tc.tile_pool(name="dram", bufs=1, space="DRAM"))
            pers = ctx.enter_context(tc.tile_pool(name="pers", bufs=1))
            sb2 = ctx.enter_context(tc.tile_pool(name="sb2", bufs=2))
            sb3 = ctx.enter_context(tc.tile_pool(name="sb3", bufs=3))
            spool = ctx.enter_context(tc.tile_pool(name="spool", bufs=4))
            gpool = ctx.enter_context(tc.tile_pool(name="gpool", bufs=GS_BUFS))
            # GRP concurrent per-tile accumulation chains (1 PSUM bank each);
            # extra slots pipeline the next group's chains. Remaining banks
            # host the transient psums (epilogue GEMMs, pooling, FC).
            n_gemm = 1 if GRP > 5 else 2
            abufs = int(os.environ.get("K_ABUFS", "0")) or (8 - n_gemm)
            agg_ps = ctx.enter_context(tc.tile_pool(name="agg_ps", bufs=abufs,
                                                    space="PSUM"))
            gemm_ps = ctx.enter_context(tc.tile_pool(name="gemm_ps",
                                                     bufs=n_gemm,
                                                     space="PSUM"))

            # ---------- DRAM intermediates ----------
            G2_shard = dram.tile([NPC, 128], bf16, name="G2_shard")
            G3_shard = dram.tile([NPC, 128], bf16, name="G3_shard")
            pool_in = dram.tile([64, F3], f32, name="pool_in")

            # ---------- persistent SBUF ----------
            # split the big idx preload so early gathers can start before
            # the whole table has landed (subtile deps gate per-slice)
            idx_sb = pers.tile([P, NCHUNK * 8], dt.int16, name="idx_sb")
            qc = cdiv(NCHUNK, 8)
            for i in range(8):
                lo, hi = i * qc * 8, min(NCHUNK, (i + 1) * qc) * 8
                if lo < hi:
                    nc.sync.dma_start(idx_sb[:, lo:hi], idx16[:, lo:hi])
            if sbuild == "ts":
                dstf_sb = pers.tile([P, NCHUNK], f32, name="dstf_sb")
                wf_sb = pers.tile([P, NCHUNK], f32, name="wf_sb")
                nc.sync.dma_start(dstf_sb[:], dstf32[:])
                nc.sync.dma_start(wf_sb[:], wf32[:])
            else:
                dstf_sb = pers.tile([P, NCHUNK], bf16, name="dstf_sb")
                wf_sb = pers.tile([P, NCHUNK], bf16, name="wf_sb")
                nc.sync.dma_start(dstf_sb[:], dstf[:])
                nc.sync.dma_start(wf_sb[:], wf[:])
            iota_sb = pers.tile([P, GRP * P], bf16, name="iota_sb")
            ident64_sb = pers.tile([64, 64], f32, name="ident64_sb")
            cinv_sb = pers.tile([G, 1], f32, name="cinv_sb")
            batchf_sb = pers.tile([P, T], f32, name="batchf_sb")
            nc.sync.dma_start(iota_sb[:], iota_in[:])
            nc.sync.dma_start(ident64_sb[:], ident64_in[:])
            nc.sync.dma_start(cinv_sb[:], cinv_in[:])
            nc.sync.dma_start(batchf_sb[:], batchf[:])
            W1_sb = pers.tile([F0, F1], bf16, name="W1_sb")
            W2_sb = pers.tile([F1, F2], bf16, name="W2_sb")
            W3_sb = pers.tile([F2, 128], bf16, name="W3_sb")
            Wfc_sb = pers.tile([F3, 1], f32, name="Wfc_sb")
            nc.sync.dma_start(W1_sb[:], W1_in[:])
            nc.sync.dma_start(W2_sb[:], W2_in[:])
            nc.sync.dma_start(W3_sb[:], W3_in[:])
            nc.sync.dma_start(Wfc_sb[:], Wfc_in[:])
            b1_sb = pers.tile([F1, 1], f32, name="b1_sb")
            b2_sb = pers.tile([F2, 1], f32, name="b2_sb")
            bfc_sb = pers.tile([64, 1], f32, name="bfc_sb")
            nc.sync.dma_start(b1_sb[:], b1_in[:])
            nc.sync.dma_start(b2_sb[:], b2_in[:])
            nc.sync.dma_start(bfc_sb[:], bfc_in[:])
            pool_sb = pers.tile([64, F3], f32, name="pool_sb")
            nc.vector.memset(pool_sb[:], 0.0)

            # ================= aggregation layer ==========================
            def agg_layer(li):
                """li=0: consume xdup, produce X2T->G2_shard (agg then W1,W2)
                   li=1: consume G2_full, produce X3T->G3_shard (transposed)
                   li=2: consume G3_full, produce pooled partials (normal)."""
                g_src = (xdup, G2_full, G3_full)[li]
                FI = (64, 128, 64)[li]       # real feature width of G rows
                transposed = li != 2
                W = GRP * P                  # group-relative position width

                def epilogue(g, apsg, aps):
                    if li == 0:
                        # xa = (A^T x)^T; h1 = relu(W1^T xa + b1);
                        # G2 = h1^T @ W2  (group-wide until the W2 GEMM)
                        stage = sb3.tile([P, GRP * F2], bf16, name="g2st",
                                         tag="g2st")
                        xa = spool.tile([F0, W], bf16, name="xa", tag="xa")
                        nc.scalar.copy(xa[:], apsg[:])
                        ps1 = gemm_ps.tile([F1, W], f32, name="ps1",
                                           tag="gps")
                        nc.tensor.matmul(ps1[:], lhsT=W1_sb[:], rhs=xa[:],
                                         start=True, stop=True)
                        x2t = spool.tile([F1, W], bf16, name="x2t",
                                         tag="x2t")
                        nc.scalar.activation(x2t[:], ps1[:], Act.Relu,
                                             bias=b1_sb[:, 0:1])
                        for j in range(GRP):
                            ps2 = gemm_ps.tile([P, F2], f32, name="ps2",
                                               tag="gps")
                            nc.tensor.matmul(ps2[:],
                                             lhsT=x2t[:, j * P:(j + 1) * P],
                                             rhs=W2_sb[:],
                                             start=True, stop=True)
                            nc.scalar.copy(stage[:, j * F2:(j + 1) * F2],
                                           ps2[:])
                        rows = G2_shard[g * GR:(g + 1) * GR, :]
                        nc.sync.dma_start(
                            rows.rearrange("(p j) f -> p j f", j=GRP),
                            stage[:].rearrange("p (j f) -> p j f", j=GRP))
                    elif li == 1:
                        stage = sb3.tile([P, GRP * 128], bf16, name="g3st",
                                         tag="g3st")
                        x3t = spool.tile([F2, W], bf16, name="x3t",
                                         tag="x3t")
                        nc.scalar.activation(x3t[:], apsg[:], Act.Relu,
                                             bias=b2_sb[:, 0:1])
                        for j in range(GRP):
                            ps3 = gemm_ps.tile([P, 128], f32, name="ps3",
                                               tag="gps")
                            nc.tensor.matmul(ps3[:],
                                             lhsT=x3t[:, j * P:(j + 1) * P],
                                             rhs=W3_sb[:],
                                             start=True, stop=True)
                            nc.scalar.copy(stage[:, j * 128:(j + 1) * 128],
                                           ps3[:])
                        rows = G3_shard[g * GR:(g + 1) * GR, :]
                        nc.sync.dma_start(
                            rows.rearrange("(p j) f -> p j f", j=GRP),
                            stage[:].rearrange("p (j f) -> p j f", j=GRP))
                    else:
                        pp = gemm_ps.tile([64, F3], f32, name="pp", tag="gps")
                        for j in range(GRP):
                            x4 = spool.tile([P, F3], bf16, name="x4", tag="x4")
                            nc.scalar.activation(x4[:], aps[j][:], Act.Relu)
                            t = g * GRP + j
                            Bt = spool.tile([P, 64], bf16, name="Bt", tag="Bt")
                            nc.vector.tensor_scalar(
                                Bt[:], iota_sb[:, :64],
                                batchf_sb[:, t:t + 1], None, Alu.is_equal)
                            nc.tensor.matmul(pp[:], lhsT=Bt[:], rhs=x4[:],
                                             start=(j == 0),
                                             stop=(j == GRP - 1))
                        nc.vector.tensor_tensor(out=pool_sb[:],
                                                in0=pool_sb[:], in1=pp[:],
                                                op=Alu.add)

                pend = []
                edelay = int(os.environ.get("K_EDELAY", "1"))
                for g0 in range(0, NGRP, GM):
                    gs = list(range(g0, min(g0 + GM, NGRP)))
                    # accumulation chains: one W-wide chain per group for
                    # transposed layers; GRP narrow chains for the last layer.
                    chains = {}
                    fcol = {}
                    lcol = {}
                    for g in gs:
                        if transposed:
                            chains[g] = (agg_ps.tile([FI, W], f32,
                                                     name="apsT", tag="aps"),
                                         None)
                        else:
                            chains[g] = (None,
                                         [agg_ps.tile([P, F3], f32,
                                                      name="aps", tag="aps")
                                          for _ in range(GRP)])
                        bs = [b for b in range(NBLK) if CHTS[(b, g)] > 0]
                        assert bs, g
                        fcol[g] = CBASE[(bs[0], g)]
                        lcol[g] = CBASE[(bs[-1], g)] + CHTS[(bs[-1], g)] - 1
                    for b in range(NBLK):
                        mcht = sum(CHTS[(b, g)] for g in gs)
                        if mcht == 0:
                            continue
                        cb0 = CBASE[(b, gs[0])]   # gs contiguous in chunk cols
                        gb = gpool.tile([P, SEGM, 128], bf16,
                                        name="gb", tag="gb")
                        if "gather" not in skips:
                            nc.gpsimd.dma_gather(
                                gb[:, :mcht, :],
                                g_src[b * BLK:(b + 1) * BLK, :],
                                idx_sb[:, cb0 * 8:(cb0 + mcht) * 8],
                                mcht * P, mcht * P, 128, single_packet=False,
                                queue_num=next_q())
                        S = spool.tile([P, SEGM, W], bf16,
                                       name="S", tag="S", bufs=S_BUFS)
                        if "svec" not in skips:
                            wcol = wf_sb[:, cb0:cb0 + mcht]
                            gview = gb[:, :mcht, :FI]
                            nc.vector.tensor_tensor(
                                out=gview, in0=gview,
                                in1=wcol[:, :, None].broadcast_to(
                                    [P, mcht, FI]),
                                op=Alu.mult)
                            nc.vector.tensor_tensor(
                                out=S[:, :mcht, :],
                                in0=iota_sb[:, None, :].broadcast_to(
                                    [P, mcht, W]),
                                in1=dstf_sb[:, cb0:cb0 + mcht]
                                [:, :, None].broadcast_to([P, mcht, W]),
                                op=Alu.is_equal)
                        if "mm" in skips:
                            continue
                        for g in gs:
                            apsg, aps = chains[g]
                            cg0 = CBASE[(b, g)]
                            for c in range(cg0, cg0 + CHTS[(b, g)]):
                                k = c - cb0
                                first = c == fcol[g]
                                last = c == lcol[g]
                                if transposed:
                                    nc.tensor.matmul(
                                        apsg[:], lhsT=gb[:, k, :FI],
                                        rhs=S[:, k, :],
                                        start=first, stop=last)
                                else:
                                    for j in range(GRP):
                                        nc.tensor.matmul(
                                            aps[j][:],
                                            lhsT=S[:, k, j * P:(j + 1) * P],
                                            rhs=gb[:, k, :FI],
                                            start=first, stop=last)
                    # ---- epilogue, delayed so the next groups' chunk
                    # matmuls sit between dependent epilogue steps in the
                    # in-order engine queues (no PE/ACT ping-pong stalls).
                    if "mm" in skips:
                        continue
                    for g in gs:
                        pend.append((g,) + chains[g])
                        if len(pend) > edelay:
                            epilogue(*pend.pop(0))
                while pend:
                    epilogue(*pend.pop(0))

            # ================= execution =================
            STAGES = ["prep", "agg1pre", "agg1", "agg2pre", "agg2",
                      "agg3", "full"]
            cut = os.environ.get("K_CUT", "full") or "full"
            cut_idx = STAGES.index(cut)

            reps = int(os.environ.get("K_REPS", "1"))
            for _rep in range(reps):
                G2_full = dram.tile([TOTAL, 128], bf16,
                                    name=f"G2_full_{_rep}",
                                    addr_space="Shared")
                G3_full = dram.tile([TOTAL, 128], bf16,
                                    name=f"G3_full_{_rep}",
                                    addr_space="Shared")
                pool_out = dram.tile([64, F3], f32, name=f"pool_out_{_rep}",
                                     addr_space="Shared")
                if _rep > 0:
                    nc.vector.memset(pool_sb[:], 0.0)
                if cut_idx >= 1:
                    agg_layer(0)
                if cut_idx >= 2:
                    nc.gpsimd.collective_compute(
                        "AllGather", Alu.bypass, ins=[G2_shard.opt()],
                        outs=[G2_full.opt()], replica_groups=rg)
                if cut_idx >= 3:
                    agg_layer(1)
                if cut_idx >= 4:
                    nc.gpsimd.collective_compute(
                        "AllGather", Alu.bypass, ins=[G3_shard.opt()],
                        outs=[G3_full.opt()], replica_groups=rg)
                if cut_idx >= 5:
                    agg_layer(2)
                if cut_idx < 6:
                    continue

                # ---- pool + FC ----
                nc.sync.dma_start(pool_in[:], pool_sb[:])
                nc.gpsimd.collective_compute(
                    "AllReduce", Alu.add, ins=[pool_in.opt()],
                    outs=[pool_out.opt()], replica_groups=rg)
                pr = sb2.tile([64, F3], f32, name="pr", tag="pr")
                nc.sync.dma_start(pr[:], pool_out[:])
                tsp = gemm_ps.tile([F3, 64], f32, name="tsp", tag="gps")
                nc.tensor.transpose(tsp[:], pr[:], ident64_sb[:])
                sT = sb2.tile([F3, 64], f32, name="sT", tag="sT")
                nc.vector.tensor_copy(sT[:], tsp[:])
                fps = gemm_ps.tile([64, 1], f32, name="fps", tag="gps")
                nc.tensor.matmul(fps[:], lhsT=sT[:], rhs=Wfc_sb[:], start=True,
                                 stop=True)
                res = sb2.tile([64, 1], f32, name="res", tag="res")
                nc.vector.tensor_scalar(res[:], fps[:], cinv_sb[:], bfc_sb[:],
                                        Alu.mult, op1=Alu.add)
                nc.sync.dma_start(out_t[:], res[:])

    nc.compile()
    return nc


# --------------------------------------------------------------------------
# Entry point
# --------------------------------------------------------------------------

_PROGRAM_CACHE = {}


def kernel(x, src, dst, edge_weight, batch, W1, b1, W2, b2, W3, b3, Wfc, bfc):
    from concourse.bass_utils import run_bass_kernel_spmd

    cfg = Cfg(**FULL_CFG)
    per_core = host_prep(x, src, dst, edge_weight, batch, W1, b1, W2, b2, W3,
                         b3, Wfc, bfc, cfg)
    key = (tuple(sorted(cfg.CHTS.items())), cfg.HAS_B3)
    if key not in _PROGRAM_CACHE:
        _PROGRAM_CACHE[key] = build_program(cfg)
    nc = _PROGRAM_CACHE[key]
    res = run_bass_kernel_spmd(nc, per_core, list(range(cfg.NC)))
    out = np.asarray(res.results[0]["out"], np.float32).reshape(cfg.G, 1)
    return out


# revision 34
# speedup vs baseline: 1.0229x; 1.0073x over previous
"""Trainium2 Bass kernel for EnhancedGNN (3x GCNConv + mean-pool + FC), v6.

Self-contained: host-side sharding/layout prep + SPMD Bass/Tile program on 8
NeuronCores. See bottom for the `kernel(**inputs)` entry point.

Design (measured best on HW; ~3.3-3.7 ms vs 4.73 ms for the previous version):
  - Epilogues software-pipelined one group behind the accumulation chains, so
    each group's PE/ACT ping-pong (psum->sbuf copy, W GEMMs, relu) sits
    between the NEXT group's chunk matmuls in the in-order engine queues
    instead of stalling them (measured -0.7 ms). Two-group delay is worse.
  - The idx preload is split into 8 slice DMAs so early gathers start before
    the full 7.4 MB table lands.
  - Nodes degree-balanced across 8 cores; edges bucketed by (dst core, dst
    tile group, src block); scatter-add done as per-chunk one-hot matmuls
    accumulating in PSUM (one open accumulation group per 2KB bank -- a HW
    rule; each tile group's chain owns one bank, 6 chains pipelined).
  - gemm1 eliminated by linearity: layer 1 aggregates raw x (gathered from a
    host-prepared [x|x] bf16 tensor) and applies W1 *after* aggregation in
    the epilogue (A^T(xW1) == (A^T x)W1). No G1 materialization.
  - GRP=1 (128-wide one-hot S): wide-mode S builds at W=256 double the DVE
    cost (measured DVE-bound), and per-chunk tensor_scalar builds collapse
    the pipeline (12.4 ms) despite being faster in isolation.
  - Gather grain ~10 chunks (1280 descriptors, near the 1024-desc SWDGE ring)
    with 8 gather buffers in flight: measured 2x faster than 2-3 buffers and
    than >4K-descriptor gathers (ring-overflow stalls).
  - GCN norm folded into per-edge weights on host; biases fused into the
    epilogue activations; mean-pool via one-hot batch matmul + AllReduce.
"""

import os
import sys

import numpy as np

for _p in ("/opt/trn_rl_repo", "/root/.axon_site", "/root/.axon_site/_ro/pypackages"):
    if os.path.isdir(_p) and _p not in sys.path:
        sys.path.append(_p)

import ml_dtypes

BF16 = ml_dtypes.bfloat16
P = 128


def cdiv(a, b):
    return -(-a // b)


class Cfg:
    def __init__(self, n_nodes, n_edges, nc, tiles_pc, grp, nblk, n_graphs):
        self.N = n_nodes
        self.E = n_edges
        self.NC = nc
        self.T = tiles_pc
        self.GRP = grp
        self.NBLK = nblk
        self.G = n_graphs
        self.NPC = self.T * P
        self.TOTAL = self.NC * self.NPC
        self.BLK = self.TOTAL // self.NBLK
        assert self.T % self.GRP == 0
        assert self.TOTAL % self.NBLK == 0
        assert self.BLK <= 32768
        assert self.N % self.NC == 0
        assert self.N // self.NC <= self.NPC
        self.F = (64, 64, 128, 64)  # F0(in), F1, F2, F3
        # filled by host_prep:
        self.CHTS = None      # {(g,b,j): n_chunks}
        self.CBASE = None     # {(g,b,j): first chunk col}
        self.NCHUNK = None    # total chunks
        self.CHT_MAX = None
        self.CHTB_MAX = None  # max chunks in a merged (g,b) bucket
        self.HAS_B3 = False


FULL_CFG = dict(n_nodes=100000, n_edges=3200000, nc=8, tiles_pc=98,
                grp=int(os.environ.get("K_GRP", "1")),
                nblk=4, n_graphs=64)


# --------------------------------------------------------------------------
# Host-side prep: node assignment, edge bucketing, layout arrays.
# --------------------------------------------------------------------------

def host_prep(x, src, dst, edge_weight, batch, W1, b1, W2, b2, W3, b3, Wfc,
              bfc, cfg: Cfg):
    N, E, NC, T = cfg.N, cfg.E, cfg.NC, cfg.T
    NPC, TOTAL, NBLK, BLK, GRP = cfg.NPC, cfg.TOTAL, cfg.NBLK, cfg.BLK, cfg.GRP
    NGRP = T // GRP
    GR = GRP * P
    F0 = cfg.F[0]
    x = np.ascontiguousarray(np.asarray(x, np.float32))
    src = np.asarray(src).astype(np.int64)
    dst = np.asarray(dst).astype(np.int64)
    ew = np.asarray(edge_weight, np.float32)
    batch = np.asarray(batch).astype(np.int64)

    # ---- node -> (core, tile, p) assignment, degree balanced ----
    degc = np.bincount(dst, minlength=N)
    order = np.argsort(-degc, kind="stable")
    ranks = np.arange(N)
    core_of = np.empty(N, np.int64)
    rank_in_core = np.empty(N, np.int64)
    core_of[order] = ranks % NC
    rank_in_core[order] = ranks // NC
    row = rank_in_core // T
    col = rank_in_core % T
    tile = np.where(row % 2 == 0, col, T - 1 - col)
    p_in_tile = row
    assert p_in_tile.max() < P

    # ---- self loops + GCN norm folded into edge weights (host) ----
    loop = np.arange(N, dtype=np.int64)
    src_f = np.concatenate([src, loop])
    dst_f = np.concatenate([dst, loop])
    ew_f = np.concatenate([ew, np.ones(N, np.float32)])
    deg = np.bincount(dst_f, weights=ew_f.astype(np.float64),
                      minlength=N).astype(np.float32)
    dinv = np.where(deg > 0, 1.0 / np.sqrt(deg), 0.0).astype(np.float32)
    norm = dinv[src_f] * ew_f * dinv[dst_f]

    # ---- cross-core tile matching to shrink SPMD chunk padding ----
    # Chunk counts per (tile slot, src block) are max'd over cores; tile slot
    # ids are arbitrary, and a node's src block (= src core // 2) is invariant
    # under within-core tile permutation, so align tiles with similar
    # block-count vectors across cores to shrink the max.
    if os.environ.get("K_MATCH", "1") == "1" and NBLK * 2 == NC:
        b_of_src = core_of[src_f] // 2
        v = np.bincount(
            (core_of[dst_f] * T + tile[dst_f]) * NBLK + b_of_src,
            minlength=NC * T * NBLK).reshape(NC, T, NBLK).astype(np.int64)
        inv = np.empty((NC, T), np.int64)
        order0 = np.argsort(-v[0].max(axis=1), kind="stable")
        inv[0, order0] = np.arange(T)
        for c in range(1, NC):
            unused = np.ones(T, bool)
            for slot, t0 in enumerate(order0):
                run = v[0, t0]
                cost = np.maximum(run[None, :], v[c]).sum(axis=1)
                cost[~unused] = 1 << 40
                best = int(np.argmin(cost))
                inv[c, best] = slot
                unused[best] = False
        tile = inv[core_of, tile]

    # G-row id: within (core, group): p * GRP + j so group writes are
    # per-partition contiguous
    g_i = tile // GRP
    j_i = tile % GRP
    grow = core_of * NPC + g_i * GR + p_in_tile * GRP + j_i

    # ---- edge bucketing by (dst core, group g, src block b) ----
    # dst position inside the S one-hot is group-relative (j*128 + p), so
    # tiles of a group share one bucket and one PSUM accumulation chain.
    assert GRP * P <= 512  # PSUM bank holds 512 fp32 columns
    e_core = core_of[dst_f]
    e_g = g_i[dst_f]
    e_pos = j_i[dst_f] * P + p_in_tile[dst_f]   # 0 .. GRP*128-1
    e_grow = grow[src_f]
    e_B = e_grow // BLK
    e_lidx = (e_grow % BLK).astype(np.int64)
    # block-major bucket order (g innermost): adjacent tiles' buckets for
    # the same source block are contiguous, so one dma_gather can span
    # several tiles' chunks (K_GM merge) without touching chain structure.
    key = (e_core * NBLK + e_B) * NGRP + e_g
    si = np.argsort(key, kind="stable")
    key_s = key[si]
    nbuck = NC * NGRP * NBLK
    bc = np.bincount(key_s, minlength=nbuck).reshape(NC, NBLK * NGRP)
    # per-core chunk counts must be IDENTICAL across cores for SPMD (one
    # program): use per-bucket max over cores.
    chts_flat = cdiv(bc, P).max(axis=0)  # [NBLK*NGRP]
    cbase_flat = np.zeros(chts_flat.size + 1, np.int64)
    np.cumsum(chts_flat, out=cbase_flat[1:])
    nchunk = int(cbase_flat[-1])
    cfg.NCHUNK = nchunk
    cfg.CHT_MAX = int(chts_flat.max())
    CHTS = {}
    CBASE = {}
    for b in range(NBLK):
        for g in range(NGRP):
            f = b * NGRP + g
            CHTS[(b, g)] = int(chts_flat[f])
            CBASE[(b, g)] = int(cbase_flat[f])
    cfg.CHTS = CHTS
    cfg.CBASE = CBASE
    cfg.CHTB_MAX = cfg.CHT_MAX

    # ---- slot assignment within buckets ----
    starts = np.zeros(nbuck + 1, np.int64)
    np.cumsum(bc.reshape(-1), out=starts[1:])
    slot = np.arange(E + N) - starts[key_s]
    core_b = key_s // (NBLK * NGRP)
    buck = key_s % (NBLK * NGRP)

    # idx / dstf / wf arrays, chunk-column layout
    idx_arr = np.zeros((NC, nchunk * P), np.int16)
    dstf = np.full((NC, P, nchunk), -1.0, np.float32)
    wff = np.zeros((NC, P, nchunk), np.float32)
    ccol = cbase_flat[buck] + slot // P     # global chunk column
    pp = slot % P
    idx_arr[core_b, ccol * P + pp] = e_lidx[si].astype(np.int16)
    dstf[core_b, pp, ccol] = e_pos[si].astype(np.float32)
    wff[core_b, pp, ccol] = norm[si]

    # 16-wrap the indices per chunk: slot e of chunk c -> [e%16, c*8 + e//16],
    # replicated x8 along partitions.
    idx16 = idx_arr.reshape(NC, nchunk, 8, 16).transpose(0, 3, 1, 2)
    idx16 = idx16.reshape(NC, 16, nchunk * 8)
    idx16 = np.ascontiguousarray(
        np.broadcast_to(idx16[:, None, :, :], (NC, 8, 16, nchunk * 8))
        .reshape(NC, P, nchunk * 8))

    # ---- batch one-hot source values (per tile col), pad -> -1 ----
    batchf = np.full((NC, P, T), -1.0, np.float32)
    batchf[core_of, p_in_tile, tile] = batch.astype(np.float32)

    # ---- features as [x|x] bf16 rows in grow order ----
    xdup = np.zeros((TOTAL, 2 * F0), np.float32)
    xdup[grow, :F0] = x
    xdup[grow, F0:] = x

    # ---- constants ----
    iota = np.tile(np.arange(GRP * P, dtype=np.float32)[None, :], (P, 1))
    ident64 = np.eye(64, dtype=np.float32)
    cnts = np.maximum(np.bincount(batch, minlength=cfg.G).astype(np.float32),
                      1.0)
    cinv = (1.0 / cnts).reshape(cfg.G, 1)

    W1 = np.asarray(W1, np.float32)
    W2 = np.asarray(W2, np.float32)
    W3 = np.asarray(W3, np.float32)
    b1 = np.asarray(b1, np.float32).reshape(-1)
    b2 = np.asarray(b2, np.float32).reshape(-1)
    b3 = np.asarray(b3, np.float32).reshape(-1)
    cfg.HAS_B3 = bool(np.any(b3 != 0.0))
    assert not cfg.HAS_B3, "nonzero b3 not supported in this kernel version"
    W3dup = np.concatenate([W3, W3], axis=1)          # [128, 128]

    xdup_bf = xdup.astype(BF16)
    per_core = []
    for c in range(NC):
        m = {
            "xdup": xdup_bf,
            "idx16": np.ascontiguousarray(idx16[c]),
            "dstf": np.ascontiguousarray(dstf[c]).astype(BF16),
            "wf": np.ascontiguousarray(wff[c]).astype(BF16),
            "dstf32": np.ascontiguousarray(dstf[c]),
            "wf32": np.ascontiguousarray(wff[c]),
            "batchf": np.ascontiguousarray(batchf[c]),
            "iota": iota.astype(BF16),
            "ident64": ident64,
            "cinv": cinv,
            "W1": W1.astype(BF16),
            "W2": W2.astype(BF16),
            "W3dup": W3dup.astype(BF16),
            "Wfc": np.asarray(Wfc, np.float32).reshape(cfg.F[3], 1),
            "b1c": b1.reshape(cfg.F[1], 1),
            "b2c": b2.reshape(cfg.F[2], 1),
            "bfcr": np.full((64, 1), np.float32(np.asarray(bfc).reshape(-1)[0])),
        }
        per_core.append(m)
    return per_core


# --------------------------------------------------------------------------
# Bass/Tile SPMD program
# --------------------------------------------------------------------------

def build_program(cfg: Cfg):
    import concourse.bacc as bacc
    import concourse.mybir as mybir
    import concourse.tile as tile

    dt = mybir.dt
    f32 = dt.float32
    bf16 = dt.bfloat16
    Alu = mybir.AluOpType
    Act = mybir.ActivationFunctionType

    NC, T, GRP, NBLK = cfg.NC, cfg.T, cfg.GRP, cfg.NBLK
    NPC, TOTAL, BLK = cfg.NPC, cfg.TOTAL, cfg.BLK
    G = cfg.G
    F0, F1, F2, F3 = cfg.F
    NGRP = T // GRP
    GR = GRP * P
    NCHUNK = cfg.NCHUNK
    CHTS, CBASE = cfg.CHTS, cfg.CBASE
    CHTB_MAX = cfg.CHTB_MAX
    GM = int(os.environ.get("K_GM", "1"))   # tiles merged per dma_gather
    SEGM = max(
        sum(CHTS[(b, g)] for g in range(g0, min(g0 + GM, NGRP)))
        for g0 in range(0, NGRP, GM) for b in range(NBLK))
    GS_BUFS = int(os.environ.get("K_GBUFS", "8"))
    S_BUFS = int(os.environ.get("K_SBUFS", "8"))
    sbuild = os.environ.get("K_SBUILD", "wide")
    skips = set(filter(None, os.environ.get("K_SKIP", "").split(",")))

    nq = int(os.environ.get("K_QUEUES", "4"))
    scratch = int(os.environ.get("K_SCRATCH", "16384"))
    nc = bacc.Bacc("TRN2", target_bir_lowering=False, debug=False,
                   enable_asserts=False, num_devices=NC,
                   num_swdge_queues=nq, dynamic_dma_scratch_size=scratch)
    _qctr = [0]

    def next_q():
        q = _qctr[0] % nq
        _qctr[0] += 1
        return q

    def inp(name, shape, dtype=f32):
        return nc.dram_tensor(name, list(shape), dtype, kind="ExternalInput")

    xdup = inp("xdup", (TOTAL, 128), bf16)
    idx16 = inp("idx16", (P, NCHUNK * 8), dt.int16)
    dstf = inp("dstf", (P, NCHUNK), bf16)
    wf = inp("wf", (P, NCHUNK), bf16)
    dstf32 = inp("dstf32", (P, NCHUNK))
    wf32 = inp("wf32", (P, NCHUNK))
    batchf = inp("batchf", (P, T))
    iota_in = inp("iota", (P, GRP * P), bf16)
    ident64_in = inp("ident64", (64, 64))
    cinv_in = inp("cinv", (G, 1))
    W1_in = inp("W1", (F0, F1), bf16)
    W2_in = inp("W2", (F1, F2), bf16)
    W3_in = inp("W3dup", (F2, 128), bf16)
    Wfc_in = inp("Wfc", (F3, 1))
    b1_in = inp("b1c", (F1, 1))
    b2_in = inp("b2c", (F2, 1))
    bfc_in = inp("bfcr", (64, 1))
    out_t = nc.dram_tensor("out", [64, 1], f32, kind="ExternalOutput")

    rg = [list(range(NC))]

    with tile.TileContext(nc) as tc:
        import contextlib
        ctx = contextlib.ExitStack()
        with ctx:
            dram = ctx.enter_context(tc.tile_pool(name="dram", bufs=1, space="DRAM"))
            pers = ctx.enter_context(tc.tile_pool(name="pers", bufs=1))
            sb2 = ctx.enter_context(tc.tile_pool(name="sb2", bufs=2))
            sb3 = ctx.enter_context(tc.tile_pool(name="sb3", bufs=3))
            spool = ctx.enter_context(tc.tile_pool(name="spool", bufs=4))
            gpool = ctx.enter_context(tc.tile_pool(name="gpool", bufs=GS_BUFS))
            # GRP concurrent per-tile accumulation chains (1 PSUM bank each);
            # extra slots pipeline the next group's chains. Remaining banks
            # host the transient psums (epilogue GEMMs, pooling, FC).
            n_gemm = 1 if GRP > 5 else 2
            abufs = int(os.environ.get("K_ABUFS", "0")) or (8 - n_gemm)
            agg_ps = ctx.enter_context(tc.tile_pool(name="agg_ps", bufs=abufs,
                                                    space="PSUM"))
            gemm_ps = ctx.enter_context(tc.tile_pool(name="gemm_ps",
                                                     bufs=n_gemm,
                                                     space="PSUM"))

            # ---------- DRAM intermediates ----------
            G2_shard = dram.tile([NPC, 128], bf16, name="G2_shard")
            G3_shard = dram.tile([NPC, 128], bf16, name="G3_shard")
            pool_in = dram.tile([64, F3], f32, name="pool_in")

            # ---------- persistent SBUF ----------
            # split the big idx preload so early gathers can start before
            # the whole table has landed (subtile deps gate per-slice)
            idx_sb = pers.tile([P, NCHUNK * 8], dt.int16, name="idx_sb")
            qc = cdiv(NCHUNK, 8)
            for i in range(8):
                lo, hi = i * qc * 8, min(NCHUNK, (i + 1) * qc) * 8
                if lo < hi:
                    nc.sync.dma_start(idx_sb[:, lo:hi], idx16[:, lo:hi])
            if sbuild == "ts":
                dstf_sb = pers.tile([P, NCHUNK], f32, name="dstf_sb")
                wf_sb = pers.tile([P, NCHUNK], f32, name="wf_sb")
                nc.sync.dma_start(dstf_sb[:], dstf32[:])
                nc.sync.dma_start(wf_sb[:], wf32[:])
            else:
                dstf_sb = pers.tile([P, NCHUNK], bf16, name="dstf_sb")
                wf_sb = pers.tile([P, NCHUNK], bf16, name="wf_sb")
                nc.sync.dma_start(dstf_sb[:], dstf[:])
                nc.sync.dma_start(wf_sb[:], wf[:])
            iota_sb = pers.tile([P, GRP * P], bf16, name="iota_sb")
            ident64_sb = pers.tile([64, 64], f32, name="ident64_sb")
            cinv_sb = pers.tile([G, 1], f32, name="cinv_sb")
            batchf_sb = pers.tile([P, T], f32, name="batchf_sb")
            nc.sync.dma_start(iota_sb[:], iota_in[:])
            nc.sync.dma_start(ident64_sb[:], ident64_in[:])
            nc.sync.dma_start(cinv_sb[:], cinv_in[:])
            nc.sync.dma_start(batchf_sb[:], batchf[:])
            W1_sb = pers.tile([F0, F1], bf16, name="W1_sb")
            W2_sb = pers.tile([F1, F2], bf16, name="W2_sb")
            W3_sb = pers.tile([F2, 128], bf16, name="W3_sb")
            Wfc_sb = pers.tile([F3, 1], f32, name="Wfc_sb")
            nc.sync.dma_start(W1_sb[:], W1_in[:])
            nc.sync.dma_start(W2_sb[:], W2_in[:])
            nc.sync.dma_start(W3_sb[:], W3_in[:])
            nc.sync.dma_start(Wfc_sb[:], Wfc_in[:])
            b1_sb = pers.tile([F1, 1], f32, name="b1_sb")
            b2_sb = pers.tile([F2, 1], f32, name="b2_sb")
            bfc_sb = pers.tile([64, 1], f32, name="bfc_sb")
            nc.sync.dma_start(b1_sb[:], b1_in[:])
            nc.sync.dma_start(b2_sb[:], b2_in[:])
            nc.sync.dma_start(bfc_sb[:], bfc_in[:])
            pool_sb = pers.tile([64, F3], f32, name="pool_sb")
            nc.vector.memset(pool_sb[:], 0.0)

            # ================= aggregation layer ==========================
            def agg_layer(li):
                """li=0: consume xdup, produce X2T->G2_shard (agg then W1,W2)
                   li=1: consume G2_full, produce X3T->G3_shard (transposed)
                   li=2: consume G3_full, produce pooled partials (normal)."""
                g_src = (xdup, G2_full, G3_full)[li]
                FI = (64, 128, 64)[li]       # real feature width of G rows
                transposed = li != 2
                W = GRP * P                  # group-relative position width

                def epilogue(g, apsg, aps):
                    if li == 0:
                        # xa = (A^T x)^T; h1 = relu(W1^T xa + b1);
                        # G2 = h1^T @ W2  (group-wide until the W2 GEMM)
                        stage = sb3.tile([P, GRP * F2], bf16, name="g2st",
                                         tag="g2st")
                        xa = spool.tile([F0, W], bf16, name="xa", tag="xa")
                        nc.scalar.copy(xa[:], apsg[:])
                        ps1 = gemm_ps.tile([F1, W], f32, name="ps1",
                                           tag="gps")
                        nc.tensor.matmul(ps1[:], lhsT=W1_sb[:], rhs=xa[:],
                                         start=True, stop=True)
                        x2t = spool.tile([F1, W], bf16, name="x2t",
                                         tag="x2t")
                        nc.scalar.activation(x2t[:], ps1[:], Act.Relu,
                                             bias=b1_sb[:, 0:1])
                        for j in range(GRP):
                            ps2 = gemm_ps.tile([P, F2], f32, name="ps2",
                                               tag="gps")
                            nc.tensor.matmul(ps2[:],
                                             lhsT=x2t[:, j * P:(j + 1) * P],
                                             rhs=W2_sb[:],
                                             start=True, stop=True)
                            nc.scalar.copy(stage[:, j * F2:(j + 1) * F2],
                                           ps2[:])
                        rows = G2_shard[g * GR:(g + 1) * GR, :]
                        nc.sync.dma_start(
                            rows.rearrange("(p j) f -> p j f", j=GRP),
                            stage[:].rearrange("p (j f) -> p j f", j=GRP))
                    elif li == 1:
                        stage = sb3.tile([P, GRP * 128], bf16, name="g3st",
                                         tag="g3st")
                        x3t = spool.tile([F2, W], bf16, name="x3t",
                                         tag="x3t")
                        nc.scalar.activation(x3t[:], apsg[:], Act.Relu,
                                             bias=b2_sb[:, 0:1])
                        for j in range(GRP):
                            ps3 = gemm_ps.tile([P, 128], f32, name="ps3",
                                               tag="gps")
                            nc.tensor.matmul(ps3[:],
                                             lhsT=x3t[:, j * P:(j + 1) * P],
                                             rhs=W3_sb[:],
                                             start=True, stop=True)
                            nc.scalar.copy(stage[:, j * 128:(j + 1) * 128],
                                           ps3[:])
                        rows = G3_shard[g * GR:(g + 1) * GR, :]
                        nc.sync.dma_start(
                            rows.rearrange("(p j) f -> p j f", j=GRP),
                            stage[:].rearrange("p (j f) -> p j f", j=GRP))
                    else:
                        pp = gemm_ps.tile([64, F3], f32, name="pp", tag="gps")
                        for j in range(GRP):
                            x4 = spool.tile([P, F3], bf16, name="x4", tag="x4")
                            nc.scalar.activation(x4[:], aps[j][:], Act.Relu)
                            t = g * GRP + j
                            Bt = spool.tile([P, 64], bf16, name="Bt", tag="Bt")
                            nc.vector.tensor_scalar(
                                Bt[:], iota_sb[:, :64],
                                batchf_sb[:, t:t + 1], None, Alu.is_equal)
                            nc.tensor.matmul(pp[:], lhsT=Bt[:], rhs=x4[:],
                                             start=(j == 0),
                                             stop=(j == GRP - 1))
                        nc.vector.tensor_tensor(out=pool_sb[:],
                                                in0=pool_sb[:], in1=pp[:],
                                                op=Alu.add)

                pend = []
                edelay = int(os.environ.get("K_EDELAY", "1"))
                for g0 in range(0, NGRP, GM):
                    gs = list(range(g0, min(g0 + GM, NGRP)))
                    # accumulation chains: one W-wide chain per group for
                    # transposed layers; GRP narrow chains for the last layer.
                    chains = {}
                    fcol = {}
                    lcol = {}
                    for g in gs:
                        if transposed:
                            chains[g] = (agg_ps.tile([FI, W], f32,
                                                     name="apsT", tag="aps"),
                                         None)
                        else:
                            chains[g] = (None,
                                         [agg_ps.tile([P, F3], f32,
                                                      name="aps", tag="aps")
                                          for _ in range(GRP)])
                        bs = [b for b in range(NBLK) if CHTS[(b, g)] > 0]
                        assert bs, g
                        fcol[g] = CBASE[(bs[0], g)]
                        lcol[g] = CBASE[(bs[-1], g)] + CHTS[(bs[-1], g)] - 1
                    for b in range(NBLK):
                        mcht = sum(CHTS[(b, g)] for g in gs)
                        if mcht == 0:
                            continue
                        cb0 = CBASE[(b, gs[0])]   # gs contiguous in chunk cols
                        gb = gpool.tile([P, SEGM, 128], bf16,
                                        name="gb", tag="gb")
                        if "gather" not in skips:
                            nc.gpsimd.dma_gather(
                                gb[:, :mcht, :],
                                g_src[b * BLK:(b + 1) * BLK, :],
                                idx_sb[:, cb0 * 8:(cb0 + mcht) * 8],
                                mcht * P, mcht * P, 128, single_packet=False,
                                queue_num=next_q())
                        S = spool.tile([P, SEGM, W], bf16,
                                       name="S", tag="S", bufs=S_BUFS)
                        if "svec" not in skips:
                            wcol = wf_sb[:, cb0:cb0 + mcht]
                            gview = gb[:, :mcht, :FI]
                            nc.vector.tensor_tensor(
                                out=gview, in0=gview,
                                in1=wcol[:, :, None].broadcast_to(
                                    [P, mcht, FI]),
                                op=Alu.mult)
                            nc.vector.tensor_tensor(
                                out=S[:, :mcht, :],
                                in0=iota_sb[:, None, :].broadcast_to(
                                    [P, mcht, W]),
                                in1=dstf_sb[:, cb0:cb0 + mcht]
                                [:, :, None].broadcast_to([P, mcht, W]),
                                op=Alu.is_equal)
                        if "mm" in skips:
                            continue
                        for g in gs:
                            apsg, aps = chains[g]
                            cg0 = CBASE[(b, g)]
                            for c in range(cg0, cg0 + CHTS[(b, g)]):
                                k = c - cb0
                                first = c == fcol[g]
                                last = c == lcol[g]
                                if transposed:
                                    nc.tensor.matmul(
                                        apsg[:], lhsT=gb[:, k, :FI],
                                        rhs=S[:, k, :],
                                        start=first, stop=last)
                                else:
                                    for j in range(GRP):
                                        nc.tensor.matmul(
                                            aps[j][:],
                                            lhsT=S[:, k, j * P:(j + 1) * P],
                                            rhs=gb[:, k, :FI],
                                            start=first, stop=last)
                    # ---- epilogue, delayed so the next groups' chunk
                    # matmuls sit between dependent epilogue steps in the
                    # in-order engine queues (no PE/ACT ping-pong stalls).
                    if "mm" in skips:
                        continue
                    for g in gs:
                        pend.append((g,) + chains[g])
                        if len(pend) > edelay:
                            epilogue(*pend.pop(0))
                while pend:
                    epilogue(*pend.pop(0))

            # ================= execution =================
            STAGES = ["prep", "agg1pre", "agg1", "agg2pre", "agg2",
                      "agg3", "full"]
            cut = os.environ.get("K_CUT", "full") or "full"
            cut_idx = STAGES.index(cut)

            reps = int(os.environ.get("K_REPS", "1"))
            for _rep in range(reps):
                G2_full = dram.tile([TOTAL, 128], bf16,
                                    name=f"G2_full_{_rep}",
                                    addr_space="Shared")
                G3_full = dram.tile([TOTAL, 128], bf16,
                                    name=f"G3_full_{_rep}",
                                    addr_space="Shared")
                pool_out = dram.tile([64, F3], f32, name=f"pool_out_{_rep}",
                                     addr_space="Shared")
                if _rep > 0:
                    nc.vector.memset(pool_sb[:], 0.0)
                if cut_idx >= 1:
                    agg_layer(0)
                if cut_idx >= 2:
                    nc.gpsimd.collective_compute(
                        "AllGather", Alu.bypass, ins=[G2_shard.opt()],
                        outs=[G2_full.opt()], replica_groups=rg)
                if cut_idx >= 3:
                    agg_layer(1)
                if cut_idx >= 4:
                    nc.gpsimd.collective_compute(
                        "AllGather", Alu.bypass, ins=[G3_shard.opt()],
                        outs=[G3_full.opt()], replica_groups=rg)
                if cut_idx >= 5:
                    agg_layer(2)
                if cut_idx < 6:
                    continue

                # ---- pool + FC ----
                nc.sync.dma_start(pool_in[:], pool_sb[:])
                nc.gpsimd.collective_compute(
                    "AllReduce", Alu.add, ins=[pool_in.opt()],
                    outs=[pool_out.opt()], replica_groups=rg)
                pr = sb2.tile([64, F3], f32, name="pr", tag="pr")
                nc.sync.dma_start(pr[:], pool_out[:])
                tsp = gemm_ps.tile([F3, 64], f32, name="tsp", tag="gps")
                nc.tensor.transpose(tsp[:], pr[:], ident64_sb[:])
                sT = sb2.tile([F3, 64], f32, name="sT", tag="sT")
                nc.vector.tensor_copy(sT[:], tsp[:])
                fps = gemm_ps.tile([64, 1], f32, name="fps", tag="gps")
                nc.tensor.matmul(fps[:], lhsT=sT[:], rhs=Wfc_sb[:], start=True,
                                 stop=True)
                res = sb2.tile([64, 1], f32, name="res", tag="res")
                nc.vector.tensor_scalar(res[:], fps[:], cinv_sb[:], bfc_sb[:],
                                        Alu.mult, op1=Alu.add)
                nc.sync.dma_start(out_t[:], res[:])

    nc.compile()
    return nc


# --------------------------------------------------------------------------
# Entry point
# --------------------------------------------------------------------------

_PROGRAM_CACHE = {}


def kernel(x, src, dst, edge_weight, batch, W1, b1, W2, b2, W3, b3, Wfc, bfc):
    from concourse.bass_utils import run_bass_kernel_spmd

    cfg = Cfg(**FULL_CFG)
    per_core = host_prep(x, src, dst, edge_weight, batch, W1, b1, W2, b2, W3,
                         b3, Wfc, bfc, cfg)
    key = (tuple(sorted(cfg.CHTS.items())), cfg.HAS_B3)
    if key not in _PROGRAM_CACHE:
        _PROGRAM_CACHE[key] = build_program(cfg)
    nc = _PROGRAM_CACHE[key]
    res = run_bass_kernel_spmd(nc, per_core, list(range(cfg.NC)))
    out = np.asarray(res.results[0]["out"], np.float32).reshape(cfg.G, 1)
    return out
